# revision 11
# baseline (speedup 1.0000x reference)
"""MoE-LoRA linear layer (T=16384, D=1024, E=64, R=8) on 8 Trainium2 cores.

Strategy: data-parallel over tokens (2048 tokens/core). Inside each core
everything is computed transposed (d on partitions, tokens on the free dim)
so every matmul consumes operands in their natural layout with no on-device
transposes:

  out_T[:, g] = sum_k W_k^T @ xT_k[:, g]      base GEMM, N=512 token groups
  out_T[:, b] += B_blk^T @ (mask_b * (A_blk^T @ xT[:, b]))   rank-8 LoRA

Routing is resolved on the host: each core's tokens are sorted by expert
label and cut into 256-token blocks; per block the (<=16) experts present
are packed into per-block A / B / mask tensors. The device program is thus
identical for all 8 cores (one SPMD NEFF) and all data-dependence lives in
input data. The LoRA matmul accumulates directly into the base GEMM's PSUM
tile (column sub-range), so composition costs no extra DVE work.

All inputs are packed on the host into SBUF-resident layouts so each tensor
needs one large DMA (sequencer descriptor generation, ~5ns/descriptor, was
the v1 bottleneck at 161 small DMAs). Compute in bf16 (f32 PSUM): fp32
matmul on TRN2 runs at 1/4 rate and would be hopelessly PE-bound; bf16 also
halves DMA traffic.
"""

import numpy as np
import ml_dtypes

import concourse.bacc as bacc
import concourse.mybir as mybir
from concourse import tile
from concourse.bass_utils import run_bass_kernel_spmd

T, D, E, R = 16384, 1024, 64, 8
N_CORES = 8
TPC = T // N_CORES          # tokens per core
KD = D // 128               # 8 contraction chunks
GRP = 512                   # base-GEMM token group (one PSUM bank)
NG = TPC // GRP             # 4 groups
SCALING = 1.0 / R
SLOTS = 128 // R            # experts per lora block the packed layout holds

BF16 = ml_dtypes.bfloat16

_compiled = {}              # n_blocks -> Bacc program (reused across calls)
_last_in_maps = None


def _build_nc(n_blocks: int):
    blk = TPC // n_blocks   # lora block (256 default)
    sub = GRP // blk        # lora blocks per token group
    bf = mybir.dt.bfloat16
    f32 = mybir.dt.float32

    nc = bacc.Bacc(
        "TRN2", target_bir_lowering=False, debug=False, num_devices=N_CORES
    )
    # host-packed SBUF layouts, one DMA each
    xT_d = nc.dram_tensor("xT", [NG, 128, KD, GRP], bf, kind="ExternalInput")
    w_d = nc.dram_tensor("W", [128, KD, D], bf, kind="ExternalInput")
    a_d = nc.dram_tensor("Ab", [128, n_blocks, KD, 128], bf, kind="ExternalInput")
    b_d = nc.dram_tensor("Bb", [128, n_blocks, D], bf, kind="ExternalInput")
    m_d = nc.dram_tensor("Mb", [128, n_blocks, blk], bf, kind="ExternalInput")
    bias_d = nc.dram_tensor("bias", [128, KD], f32, kind="ExternalInput")
    out_d = nc.dram_tensor("outT", [KD, 128, TPC], f32, kind="ExternalOutput")

    with tile.TileContext(nc) as tc:
        with (
            tc.tile_pool(name="consts", bufs=1) as cpool,
            tc.tile_pool(name="xa_ps", bufs=2, space="PSUM") as xa_ps,
            tc.tile_pool(name="out_ps", bufs=6, space="PSUM") as out_ps,
            tc.tile_pool(name="stage", bufs=4) as stage_pool,
        ):
            KH = KD // 2  # k-chunks per half tensor (split A/x0/W DMAs so
            #               the PE can start on the first half)
            bias_t = cpool.tile([128, KD], f32, tag="bias", name="bias_t")
            a_t = [
                cpool.tile([128, n_blocks * KH * 128], bf, tag=f"a{i}", name=f"a_t{i}")
                for i in range(2)
            ]
            x_t = [
                [
                    cpool.tile([128, KH * GRP], bf, tag=f"x{g}_{i}", name=f"x_t{g}_{i}")
                    for i in range(2)
                ]
                for g in range(NG)
            ]
            w_t = [
                cpool.tile([128, KH * D], bf, tag=f"w{i}", name=f"w_t{i}")
                for i in range(2)
            ]
            m_t = cpool.tile([128, n_blocks * blk], bf, tag="m", name="m_t")
            b_t = cpool.tile([128, n_blocks * D], bf, tag="b", name="b_t")
            warm_sb = cpool.tile([128, GRP], bf, tag="warm", name="warm_sb")

            def a_sl(b, k):
                i, kk = divmod(k, KH)
                return a_t[i][:, (b * KH + kk) * 128 : (b * KH + kk + 1) * 128]

            def x_sl(g, k, c0, c1):
                i, kk = divmod(k, KH)
                return x_t[g][i][:, kk * GRP + c0 : kk * GRP + c1]

            def w_sl(k, j):
                i, kk = divmod(k, KH)
                return w_t[i][:, kk * D + j * 128 : kk * D + (j + 1) * 128]

            # issue order == arrival order (one sequencer queue): what the
            # PE needs first goes first
            nc.sync.dma_start(bias_t[:], bias_d[:, :])
            nc.sync.dma_start(a_t[0][:], a_d[:, :, 0:KH, :])
            nc.sync.dma_start(x_t[0][0][:], xT_d[0, :, 0:KH, :])
            nc.sync.dma_start(w_t[0][:], w_d[:, 0:KH, :])
            nc.sync.dma_start(x_t[0][1][:], xT_d[0, :, KH:KD, :])
            nc.sync.dma_start(a_t[1][:], a_d[:, :, KH:KD, :])
            nc.sync.dma_start(w_t[1][:], w_d[:, KH:KD, :])
            nc.sync.dma_start(b_t[:], b_d[:, :, :])
            nc.sync.dma_start(m_t[:], m_d[:, :, :])
            for g in range(1, NG):
                nc.sync.dma_start(x_t[g][0][:], xT_d[g, :, 0:KH, :])
                nc.sync.dma_start(x_t[g][1][:], xT_d[g, :, KH:KD, :])

            # PE warm-up: throwaway matmuls on scratch while the first input
            # DMAs stream, so the HAM clock gate releases (1.2 -> 2.4 GHz)
            # before real work arrives.
            nc.vector.memset(warm_sb[:], 0.0)
            for _ in range(20):
                warm_ps = xa_ps.tile([128, GRP], f32, tag="xa", name="warm_ps")
                nc.tensor.matmul(
                    warm_ps[:],
                    lhsT=warm_sb[:, 0:128],
                    rhs=warm_sb[:],
                    start=True,
                    stop=True,
                    skip_group_check=True,
                )

            xa_m = [None] * n_blocks
            xa_p = [None] * n_blocks

            def emit_xa_half(b, half):
                # xa[slot, t] for lora block b (k-half), masked on completion
                g, h = divmod(b, sub)
                if half == 0:
                    xa_p[b] = xa_ps.tile([128, blk], f32, tag="xa", name=f"xa_p{b}")
                for kk in range(KH):
                    k = half * KH + kk
                    nc.tensor.matmul(
                        xa_p[b][:],
                        lhsT=a_sl(b, k),
                        rhs=x_sl(g, k, h * blk, (h + 1) * blk),
                        start=(k == 0),
                        stop=(k == KD - 1),
                    )
                if half == 1:
                    xm = cpool.tile([128, blk], bf, tag=f"xam{b}", name=f"xm{b}")
                    nc.vector.tensor_mul(
                        xm[:], xa_p[b][:], m_t[:, b * blk : (b + 1) * blk]
                    )
                    xa_m[b] = xm

            def emit_base(g, j, o_p, half):
                for kk in range(KH):
                    k = half * KH + kk
                    nc.tensor.matmul(
                        o_p[:],
                        lhsT=w_sl(k, j),
                        rhs=x_sl(g, k, 0, GRP),
                        start=(k == 0),
                        stop=False,
                        skip_group_check=True,
                    )

            def emit_lora_bias(g, j, o_p):
                for h in range(sub):
                    b = g * sub + h
                    nc.tensor.matmul(
                        o_p[:, h * blk : (h + 1) * blk],
                        lhsT=b_t[:, b * D + j * 128 : b * D + (j + 1) * 128],
                        rhs=xa_m[b][:],
                        start=False,
                        stop=(h == sub - 1),
                        skip_group_check=True,
                    )
                st = stage_pool.tile([128, GRP], f32, tag="st", name=f"st{g}_{j}")
                nc.vector.tensor_scalar_add(st[:], o_p[:], bias_t[:, j : j + 1])
                nc.sync.dma_start(out_d[j, :, g * GRP : (g + 1) * GRP], st[:])

            # --- group 0: k-split schedule matched to DMA arrival order ---
            # [A0,x00]   xa half-0
            for b in range(sub):
                emit_xa_half(b, 0)
            # [W0]       six j-tiles' first k-half (6 psum banks + 2 xa)
            o_p0 = {}
            for j in range(6):
                o_p0[j] = out_ps.tile([128, GRP], f32, tag="o", name=f"o_p0_{j}")
                emit_base(0, j, o_p0[j], 0)
            # [x01,A1]   xa half-1 + masks
            for b in range(sub):
                emit_xa_half(b, 1)
            # [W1]       finish the six, then j=6,7 whole
            for j in range(6):
                emit_base(0, j, o_p0[j], 1)
                emit_lora_bias(0, j, o_p0[j])
            for j in range(6, KD):
                o_p = out_ps.tile([128, GRP], f32, tag="o", name=f"o_p0_{j}")
                emit_base(0, j, o_p, 0)
                emit_base(0, j, o_p, 1)
                emit_lora_bias(0, j, o_p)
                if j == 6:
                    # group 1's xa, placed where its x tile has arrived
                    for h in range(sub):
                        emit_xa_half(sub + h, 0)
                        emit_xa_half(sub + h, 1)

            # --- groups 1..3: straight pipeline ---
            for g in range(1, NG):
                for j in range(KD):
                    o_p = out_ps.tile([128, GRP], f32, tag="o", name=f"o_p{g}_{j}")
                    emit_base(g, j, o_p, 0)
                    emit_base(g, j, o_p, 1)
                    emit_lora_bias(g, j, o_p)
                    if j == 3 and g < NG - 1:
                        # next group's xa, placed where its x tile has arrived
                        for h in range(sub):
                            emit_xa_half((g + 1) * sub + h, 0)
                            emit_xa_half((g + 1) * sub + h, 1)

    nc.compile()
    return nc


def _pick_n_blocks(labels: np.ndarray) -> int:
    for n_blocks in (8, 16, 32, 64, 128, 256):
        blk = TPC // n_blocks
        ok = True
        for c in range(N_CORES):
            sl = np.sort(labels[c * TPC : (c + 1) * TPC])
            for b in range(n_blocks):
                if len(np.unique(sl[b * blk : (b + 1) * blk])) > SLOTS:
                    ok = False
                    break
            if not ok:
                break
        if ok:
            return n_blocks
    raise ValueError("could not find a block size with <=16 experts per block")


def kernel(x, labels, W, A, B, bias):
    global _last_in_maps
    x = np.asarray(x, dtype=np.float32)
    labels_i = np.asarray(labels).astype(np.int64)
    W = np.asarray(W, dtype=np.float32)
    A = np.asarray(A, dtype=np.float32)
    B = np.asarray(B, dtype=np.float32)
    bias = np.asarray(bias, dtype=np.float32)

    n_blocks = _pick_n_blocks(labels_i)
    blk = TPC // n_blocks

    if n_blocks not in _compiled:
        _compiled[n_blocks] = _build_nc(n_blocks)
    nc = _compiled[n_blocks]

    # W[p, k, j] = W[128k+p, j]
    w_in = np.ascontiguousarray(W.reshape(KD, 128, D).transpose(1, 0, 2).astype(BF16))
    bias_in = np.ascontiguousarray(bias.reshape(KD, 128).T)  # [128, KD] f32
    B_scaled = (B * SCALING).astype(np.float32)

    in_maps = []
    perms = []
    for c in range(N_CORES):
        lc = labels_i[c * TPC : (c + 1) * TPC]
        perm = np.argsort(lc, kind="stable")
        perms.append(perm)
        ls = lc[perm]                          # sorted labels
        xs = x[c * TPC : (c + 1) * TPC][perm]  # [TPC, D] sorted tokens

        # xT[g, p, k, t] = xs[g*GRP + t, 128k + p]
        xT = np.ascontiguousarray(
            xs.astype(BF16).T.reshape(KD, 128, NG, GRP).transpose(2, 1, 0, 3)
        )

        a_in = np.zeros((128, n_blocks, KD, 128), dtype=BF16)
        b_in = np.zeros((128, n_blocks, D), dtype=BF16)
        m_in = np.zeros((128, n_blocks, blk), dtype=BF16)
        for b in range(n_blocks):
            seg = ls[b * blk : (b + 1) * blk]
            experts = np.unique(seg)
            assert len(experts) <= SLOTS
            for i, e in enumerate(experts):
                # lhsT slot: a_in[p, b, k, 8i+r] = A[e, 128k+p, r]
                a_in[:, b, :, i * R : (i + 1) * R] = A[e].reshape(KD, 128, R).transpose(
                    1, 0, 2
                )
                b_in[i * R : (i + 1) * R, b, :] = B_scaled[e]
                m_in[i * R : (i + 1) * R, b, :] = (seg == e)[None, :]

        in_maps.append(
            {
                "xT": xT,
                "W": w_in,
                "Ab": a_in,
                "Bb": b_in,
                "Mb": m_in,
                "bias": bias_in,
            }
        )

    _last_in_maps = in_maps
    res = run_bass_kernel_spmd(nc, in_maps, core_ids=list(range(N_CORES)))

    out = np.empty((T, D), dtype=np.float32)
    for c in range(N_CORES):
        o_t = res.results[c]["outT"].reshape(D, TPC)  # [d, t] sorted tokens
        out[c * TPC + perms[c]] = o_t.T
    return out


# revision 12
# speedup vs baseline: 1.0254x; 1.0254x over previous
"""MoE-LoRA linear layer (T=16384, D=1024, E=64, R=8) on 8 Trainium2 cores.

Strategy: data-parallel over tokens (2048 tokens/core). Inside each core
everything is computed transposed (d on partitions, tokens on the free dim)
so every matmul consumes operands in their natural layout with no on-device
transposes:

  out_T[:, g] = sum_k W_k^T @ xT_k[:, g]      base GEMM, N=512 token groups
  out_T[:, b] += B_blk^T @ (mask_b * (A_blk^T @ xT[:, b]))   rank-8 LoRA

Routing is resolved on the host: each core's tokens are sorted by expert
label and cut into 256-token blocks; per block the (<=16) experts present
are packed into per-block A / B / mask tensors. The device program is thus
identical for all 8 cores (one SPMD NEFF) and all data-dependence lives in
input data. The LoRA matmul accumulates directly into the base GEMM's PSUM
tile (column sub-range), so composition costs no extra DVE work.

Schedule: the first token group's x/A/W stream in four k-pair "waves"
(~1MB each) whose arrival rate matches PE consumption, with throwaway
warm-up matmuls bridging the fixed ~7.5us framework preamble so the PE
clock gate (HAM, 1.2 -> 2.4 GHz) releases early and never re-throttles.
Inputs are host-packed into SBUF-resident layouts so every DMA is
descriptor-cheap (128 contiguous rows); compute in bf16 (f32 PSUM): fp32
matmul on TRN2 runs at 1/4 rate and would be hopelessly PE-bound.
"""

import numpy as np
import ml_dtypes

import concourse.bacc as bacc
import concourse.mybir as mybir
from concourse import tile
from concourse.bass_utils import run_bass_kernel_spmd

T, D, E, R = 16384, 1024, 64, 8
N_CORES = 8
TPC = T // N_CORES          # tokens per core
KD = D // 128               # 8 contraction chunks
KQ = KD // 2                # k-pair waves for the first group
KH = KD // 2                # k-chunks per half tensor (groups 1+)
GRP = 512                   # base-GEMM token group (one PSUM bank)
NG = TPC // GRP             # 4 groups
SCALING = 1.0 / R
SLOTS = 128 // R            # experts per lora block the packed layout holds

BF16 = ml_dtypes.bfloat16

_compiled = {}              # n_blocks -> Bacc program (reused across calls)
_last_in_maps = None


def _build_nc(n_blocks: int):
    blk = TPC // n_blocks   # lora block (256 default)
    sub = GRP // blk        # lora blocks per token group
    bf = mybir.dt.bfloat16
    f32 = mybir.dt.float32

    nc = bacc.Bacc(
        "TRN2", target_bir_lowering=False, debug=False, num_devices=N_CORES
    )
    # host-packed SBUF layouts; every DMA source is contiguous per partition
    x0_d = nc.dram_tensor("x0", [KQ, 128, 2, GRP], bf, kind="ExternalInput")
    xr_d = nc.dram_tensor("xr", [NG - 1, 2, 128, KH, GRP], bf, kind="ExternalInput")
    w_d = nc.dram_tensor("W", [KQ, 128, 2, D], bf, kind="ExternalInput")
    a_d = nc.dram_tensor("Ab", [KQ, 128, n_blocks, 2, 128], bf, kind="ExternalInput")
    bf_d = nc.dram_tensor("Bf", [128, sub, D], bf, kind="ExternalInput")
    br_d = nc.dram_tensor("Br", [128, n_blocks - sub, D], bf, kind="ExternalInput")
    mf_d = nc.dram_tensor("Mf", [128, sub, blk], bf, kind="ExternalInput")
    mr_d = nc.dram_tensor("Mr", [128, n_blocks - sub, blk], bf, kind="ExternalInput")
    bias_d = nc.dram_tensor("bias", [128, KD], f32, kind="ExternalInput")
    out_d = nc.dram_tensor("outT", [KD, 128, TPC], f32, kind="ExternalOutput")

    with tile.TileContext(nc) as tc:
        with (
            tc.tile_pool(name="consts", bufs=1) as cpool,
            tc.tile_pool(name="xa_ps", bufs=2, space="PSUM") as xa_ps,
            tc.tile_pool(name="out_ps", bufs=6, space="PSUM") as out_ps,
            tc.tile_pool(name="stage", bufs=4) as stage_pool,
        ):
            bias_t = cpool.tile([128, KD], f32, tag="bias", name="bias_t")
            x0_t = [
                cpool.tile([128, 2 * GRP], bf, tag=f"x0_{q}", name=f"x0_t{q}")
                for q in range(KQ)
            ]
            xr_t = [
                [
                    cpool.tile([128, KH * GRP], bf, tag=f"x{g}_{i}", name=f"xr_t{g}_{i}")
                    for i in range(2)
                ]
                for g in range(1, NG)
            ]
            w_t = [
                cpool.tile([128, 2 * D], bf, tag=f"w{q}", name=f"w_t{q}")
                for q in range(KQ)
            ]
            a_t = [
                cpool.tile([128, n_blocks * 2 * 128], bf, tag=f"a{q}", name=f"a_t{q}")
                for q in range(KQ)
            ]
            bf_t = cpool.tile([128, sub * D], bf, tag="bf", name="bf_t")
            br_t = cpool.tile([128, (n_blocks - sub) * D], bf, tag="br", name="br_t")
            mf_t = cpool.tile([128, sub * blk], bf, tag="mf", name="mf_t")
            mr_t = cpool.tile(
                [128, (n_blocks - sub) * blk], bf, tag="mr", name="mr_t"
            )
            warm_sb = cpool.tile([128, GRP], bf, tag="warm", name="warm_sb")

            def a_sl(b, k):
                q, kk = divmod(k, 2)
                return a_t[q][:, (b * 2 + kk) * 128 : (b * 2 + kk + 1) * 128]

            def w_sl(k, j):
                q, kk = divmod(k, 2)
                return w_t[q][:, kk * D + j * 128 : kk * D + (j + 1) * 128]

            def x_sl(g, k, c0, c1):
                if g == 0:
                    q, kk = divmod(k, 2)
                    return x0_t[q][:, kk * GRP + c0 : kk * GRP + c1]
                i, kk = divmod(k, KH)
                return xr_t[g - 1][i][:, kk * GRP + c0 : kk * GRP + c1]

            def b_sl(b, j):
                if b < sub:
                    return bf_t[:, b * D + j * 128 : b * D + (j + 1) * 128]
                return br_t[:, (b - sub) * D + j * 128 : (b - sub) * D + (j + 1) * 128]

            def m_sl(b):
                if b < sub:
                    return mf_t[:, b * blk : (b + 1) * blk]
                return mr_t[:, (b - sub) * blk : (b - sub + 1) * blk]

            # issue order == arrival order (one sequencer queue): k-pair
            # waves for group 0, then lora data for group 0, then the rest
            nc.sync.dma_start(bias_t[:], bias_d[:, :])
            for q in range(KQ):
                nc.sync.dma_start(x0_t[q][:], x0_d[q, :, :, :])
                nc.sync.dma_start(a_t[q][:], a_d[q, :, :, :, :])
                nc.sync.dma_start(w_t[q][:], w_d[q, :, :, :])
            nc.sync.dma_start(bf_t[:], bf_d[:, :, :])
            nc.sync.dma_start(mf_t[:], mf_d[:, :, :])
            nc.sync.dma_start(xr_t[0][0][:], xr_d[0, 0, :, :, :])
            nc.sync.dma_start(xr_t[0][1][:], xr_d[0, 1, :, :, :])
            nc.sync.dma_start(br_t[:], br_d[:, :, :])
            nc.sync.dma_start(mr_t[:], mr_d[:, :, :])
            for g in range(2, NG):
                nc.sync.dma_start(xr_t[g - 1][0][:], xr_d[g - 1, 0, :, :, :])
                nc.sync.dma_start(xr_t[g - 1][1][:], xr_d[g - 1, 1, :, :, :])

            # PE warm-up across the fixed framework preamble
            nc.vector.memset(warm_sb[:], 0.0)
            for _ in range(13):
                warm_ps = xa_ps.tile([128, GRP], f32, tag="xa", name="warm_ps")
                nc.tensor.matmul(
                    warm_ps[:],
                    lhsT=warm_sb[:, 0:128],
                    rhs=warm_sb[:],
                    start=True,
                    stop=True,
                    skip_group_check=True,
                )

            xa_m = [None] * n_blocks
            xa_p = [None] * n_blocks

            def emit_xa(b, ks, masked):
                # xa[slot, t] for lora block b over k-chunks ks
                g, h = divmod(b, sub)
                if ks[0] == 0:
                    xa_p[b] = xa_ps.tile([128, blk], f32, tag="xa", name=f"xa_p{b}")
                for k in ks:
                    nc.tensor.matmul(
                        xa_p[b][:],
                        lhsT=a_sl(b, k),
                        rhs=x_sl(g, k, h * blk, (h + 1) * blk),
                        start=(k == 0),
                        stop=(k == KD - 1),
                    )
                if masked:
                    xm = cpool.tile([128, blk], bf, tag=f"xam{b}", name=f"xm{b}")
                    nc.vector.tensor_mul(xm[:], xa_p[b][:], m_sl(b))
                    xa_m[b] = xm

            def emit_base(g, j, o_p, ks):
                for k in ks:
                    nc.tensor.matmul(
                        o_p[:],
                        lhsT=w_sl(k, j),
                        rhs=x_sl(g, k, 0, GRP),
                        start=(k == 0),
                        stop=False,
                        skip_group_check=True,
                    )

            def emit_lora_bias(g, j, o_p):
                for h in range(sub):
                    b = g * sub + h
                    nc.tensor.matmul(
                        o_p[:, h * blk : (h + 1) * blk],
                        lhsT=b_sl(b, j),
                        rhs=xa_m[b][:],
                        start=False,
                        stop=(h == sub - 1),
                        skip_group_check=True,
                    )
                st = stage_pool.tile([128, GRP], f32, tag="st", name=f"st{g}_{j}")
                nc.vector.tensor_scalar_add(st[:], o_p[:], bias_t[:, j : j + 1])
                nc.sync.dma_start(out_d[j, :, g * GRP : (g + 1) * GRP], st[:])

            # --- group 0: wave schedule matched to DMA arrivals ---
            o_p0 = {}
            for j in range(6):
                o_p0[j] = out_ps.tile([128, GRP], f32, tag="o", name=f"o_p0_{j}")
            for q in range(KQ):
                ks = (2 * q, 2 * q + 1)
                for j in range(6):
                    emit_base(0, j, o_p0[j], ks)
                for b in range(sub):
                    emit_xa(b, ks, masked=(q == KQ - 1))
            for j in range(6):
                emit_lora_bias(0, j, o_p0[j])
            for j in range(6, KD):
                o_p = out_ps.tile([128, GRP], f32, tag="o", name=f"o_p0_{j}")
                emit_base(0, j, o_p, range(KD))
                emit_lora_bias(0, j, o_p)
            for h in range(sub):
                emit_xa(sub + h, range(KD), masked=True)

            # --- groups 1..3: straight pipeline ---
            for g in range(1, NG):
                for j in range(KD):
                    o_p = out_ps.tile([128, GRP], f32, tag="o", name=f"o_p{g}_{j}")
                    emit_base(g, j, o_p, range(KD))
                    emit_lora_bias(g, j, o_p)
                    if j == 3 and g < NG - 1:
                        # next group's xa, placed where its x tile has arrived
                        for h in range(sub):
                            emit_xa((g + 1) * sub + h, range(KD), masked=True)

    nc.compile()
    return nc


def _pick_n_blocks(labels: np.ndarray) -> int:
    for n_blocks in (8, 16, 32, 64, 128, 256):
        blk = TPC // n_blocks
        ok = True
        for c in range(N_CORES):
            sl = np.sort(labels[c * TPC : (c + 1) * TPC])
            for b in range(n_blocks):
                if len(np.unique(sl[b * blk : (b + 1) * blk])) > SLOTS:
                    ok = False
                    break
            if not ok:
                break
        if ok:
            return n_blocks
    raise ValueError("could not find a block size with <=16 experts per block")


def kernel(x, labels, W, A, B, bias):
    global _last_in_maps
    x = np.asarray(x, dtype=np.float32)
    labels_i = np.asarray(labels).astype(np.int64)
    W = np.asarray(W, dtype=np.float32)
    A = np.asarray(A, dtype=np.float32)
    B = np.asarray(B, dtype=np.float32)
    bias = np.asarray(bias, dtype=np.float32)

    n_blocks = _pick_n_blocks(labels_i)
    blk = TPC // n_blocks
    sub = GRP // blk

    if n_blocks not in _compiled:
        _compiled[n_blocks] = _build_nc(n_blocks)
    nc = _compiled[n_blocks]

    # W[q, p, kk, :] = W[128*(2q+kk)+p, :]
    w_in = np.ascontiguousarray(
        W.reshape(KQ, 2, 128, D).transpose(0, 2, 1, 3).astype(BF16)
    )
    bias_in = np.ascontiguousarray(bias.reshape(KD, 128).T)  # [128, KD] f32
    B_scaled = (B * SCALING).astype(np.float32)

    in_maps = []
    perms = []
    for c in range(N_CORES):
        lc = labels_i[c * TPC : (c + 1) * TPC]
        perm = np.argsort(lc, kind="stable")
        perms.append(perm)
        ls = lc[perm]                          # sorted labels
        xs = x[c * TPC : (c + 1) * TPC][perm]  # [TPC, D] sorted tokens

        # xt_full[k, p, g, t] = xs[g*GRP + t, 128k + p]
        xt_full = xs.astype(BF16).T.reshape(KD, 128, NG, GRP)
        x0_in = np.ascontiguousarray(
            xt_full[:, :, 0, :].reshape(KQ, 2, 128, GRP).transpose(0, 2, 1, 3)
        )
        xr_in = np.ascontiguousarray(
            xt_full[:, :, 1:, :]                      # [KD, 128, NG-1, GRP]
            .transpose(2, 0, 1, 3)                    # [NG-1, KD, 128, GRP]
            .reshape(NG - 1, 2, KH, 128, GRP)
            .transpose(0, 1, 3, 2, 4)                 # [NG-1, 2, 128, KH, GRP]
        )

        # packed per-block expert tables
        a_pack = np.zeros((128, n_blocks, KD, 128), dtype=BF16)
        b_pack = np.zeros((128, n_blocks, D), dtype=BF16)
        m_pack = np.zeros((128, n_blocks, blk), dtype=BF16)
        for b in range(n_blocks):
            seg = ls[b * blk : (b + 1) * blk]
            experts = np.unique(seg)
            assert len(experts) <= SLOTS
            for i, e in enumerate(experts):
                # lhsT slot: a_pack[p, b, k, 8i+r] = A[e, 128k+p, r]
                a_pack[:, b, :, i * R : (i + 1) * R] = A[e].reshape(
                    KD, 128, R
                ).transpose(1, 0, 2)
                b_pack[i * R : (i + 1) * R, b, :] = B_scaled[e]
                m_pack[i * R : (i + 1) * R, b, :] = (seg == e)[None, :]

        # a_in[q, p, b, kk, s] = a_pack[p, b, 2q+kk, s]
        a_in = np.ascontiguousarray(
            a_pack.reshape(128, n_blocks, KQ, 2, 128).transpose(2, 0, 1, 3, 4)
        )

        in_maps.append(
            {
                "x0": x0_in,
                "xr": xr_in,
                "W": w_in,
                "Ab": a_in,
                "Bf": np.ascontiguousarray(b_pack[:, :sub]),
                "Br": np.ascontiguousarray(b_pack[:, sub:]),
                "Mf": np.ascontiguousarray(m_pack[:, :sub]),
                "Mr": np.ascontiguousarray(m_pack[:, sub:]),
                "bias": bias_in,
            }
        )

    _last_in_maps = in_maps
    res = run_bass_kernel_spmd(nc, in_maps, core_ids=list(range(N_CORES)))

    out = np.empty((T, D), dtype=np.float32)
    for c in range(N_CORES):
        o_t = res.results[c]["outT"].reshape(D, TPC)  # [d, t] sorted tokens
        out[c * TPC + perms[c]] = o_t.T
    return out


# revision 13
# speedup vs baseline: 1.0651x; 1.0387x over previous
"""MoE-LoRA linear layer (T=16384, D=1024, E=64, R=8) on 8 Trainium2 cores.

Strategy: data-parallel over tokens (2048 tokens/core). Inside each core
everything is computed transposed (d on partitions, tokens on the free dim)
so every matmul consumes operands in their natural layout with no on-device
transposes:

  out_T[:, g] = sum_k W_k^T @ xT_k[:, g]      base GEMM, N=512 token groups
  out_T[:, b] += B_blk^T @ (mask_b * (A_blk^T @ xT[:, b]))   rank-8 LoRA

Routing is resolved on the host: each core's tokens are sorted by expert
label and cut into 256-token blocks; per block the (<=16) experts present
are packed into per-block A / B / mask tensors. The device program is thus
identical for all 8 cores (one SPMD NEFF) and all data-dependence lives in
input data. The LoRA matmul accumulates directly into the base GEMM's PSUM
tile (column sub-range), so composition costs no extra DVE work.

Schedule: the first token group's x/A/W stream as four combined k-pair
"waves" (one ~1.25MB DMA each) whose arrival rate matches PE consumption;
later groups' x and the B/mask tables arrive as single DMAs ordered by
first use. Throwaway warm-up matmuls bridge the fixed ~7.5us framework
preamble so the PE clock gate (HAM, 1.2 -> 2.4 GHz) releases early and
never re-throttles. Compute in bf16 (f32 PSUM): fp32 matmul on TRN2 runs
at 1/4 rate and would be hopelessly PE-bound.
"""

import numpy as np
import ml_dtypes

import concourse.bacc as bacc
import concourse.mybir as mybir
from concourse import tile
from concourse.bass_utils import run_bass_kernel_spmd

T, D, E, R = 16384, 1024, 64, 8
N_CORES = 8
TPC = T // N_CORES          # tokens per core
KD = D // 128               # 8 contraction chunks
KQ = KD // 2                # k-pair waves for the first group
GRP = 512                   # base-GEMM token group (one PSUM bank)
NG = TPC // GRP             # 4 groups
SCALING = 1.0 / R
SLOTS = 128 // R            # experts per lora block the packed layout holds

BF16 = ml_dtypes.bfloat16

_compiled = {}              # n_blocks -> Bacc program (reused across calls)
_last_in_maps = None


def _build_nc(n_blocks: int):
    blk = TPC // n_blocks   # lora block (256 default)
    sub = GRP // blk        # lora blocks per token group
    WV = 2 * GRP + n_blocks * 2 * 128 + 2 * D   # combined wave row: x | A | W
    LB = D + blk                                 # lora-table row per block: B | M
    bf = mybir.dt.bfloat16
    f32 = mybir.dt.float32

    nc = bacc.Bacc(
        "TRN2", target_bir_lowering=False, debug=False, num_devices=N_CORES
    )
    # host-packed SBUF layouts; every DMA source is contiguous per partition
    wv_d = nc.dram_tensor("wv", [KQ, 128, WV], bf, kind="ExternalInput")
    xr_d = nc.dram_tensor("xr", [NG - 1, 128, KD, GRP], bf, kind="ExternalInput")
    # lora tables in three pieces by first use: group0, group1, groups 2-3
    lt_shapes = [sub, sub, n_blocks - 2 * sub]
    lt_d = [
        nc.dram_tensor(f"lt{i}", [128, n * LB], bf, kind="ExternalInput")
        for i, n in enumerate(lt_shapes)
    ]
    bias_d = nc.dram_tensor("bias", [128, KD], f32, kind="ExternalInput")
    out_d = nc.dram_tensor("outT", [KD, 128, TPC], f32, kind="ExternalOutput")

    with tile.TileContext(nc) as tc:
        with (
            tc.tile_pool(name="consts", bufs=1) as cpool,
            tc.tile_pool(name="xa_ps", bufs=3, space="PSUM") as xa_ps,
            tc.tile_pool(name="out_ps", bufs=5, space="PSUM") as out_ps,
            tc.tile_pool(name="stage", bufs=4) as stage_pool,
        ):
            bias_t = cpool.tile([128, KD], f32, tag="bias", name="bias_t")
            wv_t = [
                cpool.tile([128, WV], bf, tag=f"wv{q}", name=f"wv_t{q}")
                for q in range(KQ)
            ]
            xr_t = [
                cpool.tile([128, KD * GRP], bf, tag=f"xr{g}", name=f"xr_t{g}")
                for g in range(1, NG)
            ]
            lt_t = [
                cpool.tile([128, n * LB], bf, tag=f"lt{i}", name=f"lt_t{i}")
                for i, n in enumerate(lt_shapes)
            ]
            warm_sb = cpool.tile([128, GRP], bf, tag="warm", name="warm_sb")

            A_OFF = 2 * GRP
            W_OFF = 2 * GRP + n_blocks * 2 * 128

            def a_sl(b, k):
                q, kk = divmod(k, 2)
                o = A_OFF + (b * 2 + kk) * 128
                return wv_t[q][:, o : o + 128]

            def w_sl(k, j):
                q, kk = divmod(k, 2)
                o = W_OFF + kk * D + j * 128
                return wv_t[q][:, o : o + 128]

            def x_sl(g, k, c0, c1):
                if g == 0:
                    q, kk = divmod(k, 2)
                    return wv_t[q][:, kk * GRP + c0 : kk * GRP + c1]
                return xr_t[g - 1][:, k * GRP + c0 : k * GRP + c1]

            def _lt(b):
                if b < sub:
                    return lt_t[0], b
                if b < 2 * sub:
                    return lt_t[1], b - sub
                return lt_t[2], b - 2 * sub

            def b_sl(b, j):
                t, lb = _lt(b)
                o = lb * LB + j * 128
                return t[:, o : o + 128]

            def m_sl(b):
                t, lb = _lt(b)
                o = lb * LB + D
                return t[:, o : o + blk]

            # issue order == arrival order (one sequencer queue)
            nc.sync.dma_start(bias_t[:], bias_d[:, :])
            for q in range(KQ):
                nc.sync.dma_start(wv_t[q][:], wv_d[q, :, :])
            nc.sync.dma_start(lt_t[0][:], lt_d[0][:, :])
            nc.sync.dma_start(xr_t[0][:], xr_d[0, :, :, :])
            nc.sync.dma_start(lt_t[1][:], lt_d[1][:, :])
            nc.sync.dma_start(xr_t[1][:], xr_d[1, :, :, :])
            nc.sync.dma_start(lt_t[2][:], lt_d[2][:, :])
            nc.sync.dma_start(xr_t[2][:], xr_d[2, :, :, :])

            # PE warm-up across the fixed framework preamble
            nc.vector.memset(warm_sb[:], 0.0)
            for _ in range(13):
                warm_ps = xa_ps.tile([128, GRP], f32, tag="xa", name="warm_ps")
                nc.tensor.matmul(
                    warm_ps[:],
                    lhsT=warm_sb[:, 0:128],
                    rhs=warm_sb[:],
                    start=True,
                    stop=True,
                    skip_group_check=True,
                )

            xa_m = [None] * n_blocks
            xa_p = [None] * n_blocks

            def emit_xa(b, ks, masked):
                # xa[slot, t] for lora block b over k-chunks ks
                g, h = divmod(b, sub)
                if ks[0] == 0:
                    xa_p[b] = xa_ps.tile([128, blk], f32, tag="xa", name=f"xa_p{b}")
                for k in ks:
                    nc.tensor.matmul(
                        xa_p[b][:],
                        lhsT=a_sl(b, k),
                        rhs=x_sl(g, k, h * blk, (h + 1) * blk),
                        start=(k == 0),
                        stop=(k == KD - 1),
                    )
                if masked:
                    xm = cpool.tile([128, blk], bf, tag=f"xam{b}", name=f"xm{b}")
                    nc.vector.tensor_mul(xm[:], xa_p[b][:], m_sl(b))
                    xa_m[b] = xm

            def emit_base(g, j, o_p, ks):
                for k in ks:
                    nc.tensor.matmul(
                        o_p[:],
                        lhsT=w_sl(k, j),
                        rhs=x_sl(g, k, 0, GRP),
                        start=(k == 0),
                        stop=False,
                        skip_group_check=True,
                    )

            def emit_lora_bias(g, j, o_p):
                for h in range(sub):
                    b = g * sub + h
                    nc.tensor.matmul(
                        o_p[:, h * blk : (h + 1) * blk],
                        lhsT=b_sl(b, j),
                        rhs=xa_m[b][:],
                        start=False,
                        stop=(h == sub - 1),
                        skip_group_check=True,
                    )
                st = stage_pool.tile([128, GRP], f32, tag="st", name=f"st{g}_{j}")
                nc.vector.tensor_scalar_add(st[:], o_p[:], bias_t[:, j : j + 1])
                nc.sync.dma_start(out_d[j, :, g * GRP : (g + 1) * GRP], st[:])

            # --- group 0: wave schedule matched to DMA arrivals ---
            o_p0 = {}
            for j in range(5):
                o_p0[j] = out_ps.tile([128, GRP], f32, tag="o", name=f"o_p0_{j}")
            for q in range(KQ):
                ks = (2 * q, 2 * q + 1)
                for j in range(5):
                    emit_base(0, j, o_p0[j], ks)
                for b in range(sub):
                    emit_xa(b, ks, masked=(q == KQ - 1))
            for j in range(5):
                emit_lora_bias(0, j, o_p0[j])
            for j in range(5, KD):
                o_p = out_ps.tile([128, GRP], f32, tag="o", name=f"o_p0_{j}")
                emit_base(0, j, o_p, range(KD))
                emit_lora_bias(0, j, o_p)
                if j == 5:
                    for h in range(sub):
                        emit_xa(sub + h, range(KD), masked=True)

            # --- groups 1..3: straight pipeline ---
            for g in range(1, NG):
                for j in range(KD):
                    o_p = out_ps.tile([128, GRP], f32, tag="o", name=f"o_p{g}_{j}")
                    emit_base(g, j, o_p, range(KD))
                    emit_lora_bias(g, j, o_p)
                    if j == 3 and g < NG - 1:
                        # next group's xa, placed where its x tile has arrived
                        for h in range(sub):
                            emit_xa((g + 1) * sub + h, range(KD), masked=True)

    nc.compile()
    return nc


def _pick_n_blocks(labels: np.ndarray) -> int:
    for n_blocks in (8, 16, 32, 64, 128, 256):
        blk = TPC // n_blocks
        ok = True
        for c in range(N_CORES):
            sl = np.sort(labels[c * TPC : (c + 1) * TPC])
            for b in range(n_blocks):
                if len(np.unique(sl[b * blk : (b + 1) * blk])) > SLOTS:
                    ok = False
                    break
            if not ok:
                break
        if ok:
            return n_blocks
    raise ValueError("could not find a block size with <=16 experts per block")


def kernel(x, labels, W, A, B, bias):
    global _last_in_maps
    x = np.asarray(x, dtype=np.float32)
    labels_i = np.asarray(labels).astype(np.int64)
    W = np.asarray(W, dtype=np.float32)
    A = np.asarray(A, dtype=np.float32)
    B = np.asarray(B, dtype=np.float32)
    bias = np.asarray(bias, dtype=np.float32)

    n_blocks = _pick_n_blocks(labels_i)
    blk = TPC // n_blocks
    sub = GRP // blk

    if n_blocks not in _compiled:
        _compiled[n_blocks] = _build_nc(n_blocks)
    nc = _compiled[n_blocks]

    # w_wave[q, p, kk, :] = W[128*(2q+kk)+p, :]
    w_wave = W.reshape(KQ, 2, 128, D).transpose(0, 2, 1, 3).astype(BF16)
    bias_in = np.ascontiguousarray(bias.reshape(KD, 128).T)  # [128, KD] f32
    B_scaled = (B * SCALING).astype(np.float32)

    in_maps = []
    perms = []
    for c in range(N_CORES):
        lc = labels_i[c * TPC : (c + 1) * TPC]
        perm = np.argsort(lc, kind="stable")
        perms.append(perm)
        ls = lc[perm]                          # sorted labels
        xs = x[c * TPC : (c + 1) * TPC][perm]  # [TPC, D] sorted tokens

        # xt_full[k, p, g, t] = xs[g*GRP + t, 128k + p]
        xt_full = xs.astype(BF16).T.reshape(KD, 128, NG, GRP)
        # x0 wave part [q, p, kk, t]
        x0_wave = xt_full[:, :, 0, :].reshape(KQ, 2, 128, GRP).transpose(0, 2, 1, 3)
        xr_in = np.ascontiguousarray(
            xt_full[:, :, 1:, :].transpose(2, 1, 0, 3)    # [NG-1, 128, KD, GRP]
        )

        # packed per-block expert tables
        a_pack = np.zeros((128, n_blocks, KD, 128), dtype=BF16)
        b_pack = np.zeros((128, n_blocks, D), dtype=BF16)
        m_pack = np.zeros((128, n_blocks, blk), dtype=BF16)
        for b in range(n_blocks):
            seg = ls[b * blk : (b + 1) * blk]
            experts = np.unique(seg)
            assert len(experts) <= SLOTS
            for i, e in enumerate(experts):
                # lhsT slot: a_pack[p, b, k, 8i+r] = A[e, 128k+p, r]
                a_pack[:, b, :, i * R : (i + 1) * R] = A[e].reshape(
                    KD, 128, R
                ).transpose(1, 0, 2)
                b_pack[i * R : (i + 1) * R, b, :] = B_scaled[e]
                m_pack[i * R : (i + 1) * R, b, :] = (seg == e)[None, :]

        # wave[q] = x0 | A | W  (concat along free dim)
        a_wave = a_pack.reshape(128, n_blocks, KQ, 2, 128).transpose(2, 0, 1, 3, 4)
        wv_in = np.ascontiguousarray(
            np.concatenate(
                [
                    x0_wave.reshape(KQ, 128, -1),
                    a_wave.reshape(KQ, 128, -1),
                    w_wave.reshape(KQ, 128, -1),
                ],
                axis=2,
            )
        )

        # lora tables: per block, B row | mask row
        lt_full = np.concatenate([b_pack, m_pack], axis=2)  # [128, nb, D+blk]
        lt_ins = [
            np.ascontiguousarray(lt_full[:, :sub].reshape(128, -1)),
            np.ascontiguousarray(lt_full[:, sub : 2 * sub].reshape(128, -1)),
            np.ascontiguousarray(lt_full[:, 2 * sub :].reshape(128, -1)),
        ]

        in_maps.append(
            {
                "wv": wv_in,
                "xr": xr_in,
                "lt0": lt_ins[0],
                "lt1": lt_ins[1],
                "lt2": lt_ins[2],
                "bias": bias_in,
            }
        )

    _last_in_maps = in_maps
    res = run_bass_kernel_spmd(nc, in_maps, core_ids=list(range(N_CORES)))

    out = np.empty((T, D), dtype=np.float32)
    for c in range(N_CORES):
        o_t = res.results[c]["outT"].reshape(D, TPC)  # [d, t] sorted tokens
        out[c * TPC + perms[c]] = o_t.T
    return out
